# revision 5
# baseline (speedup 1.0000x reference)
import os
os.environ["NEURON_CC_FLAGS"] = os.environ.get("NEURON_CC_FLAGS", "") + " --auto-cast none"
import numpy as np
import jax
import jax.numpy as jnp

jax.config.update("jax_default_matmul_precision", "highest")

# Problem constants (nn_BiLevelRoutingAttention): B=2, H=W=56, C=256
B, H, W, C = 2, 56, 56, 256
NH, HD = 8, 32
NW, HWIN = 7, 8          # 7x7 windows of 8x8 tokens
S_SP = H * W             # 3136 spatial tokens
S = S_SP + 1             # + cls token
BLK = HWIN * HWIN        # 64 tokens per block
N_BLK = 50               # 49 spatial blocks + cls block
SCALE = C ** (-0.5)      # 1/16
QPAD = 832               # 13 blocks worth of queries per core (padded)
Q_LO = (0, 832, 1664, 2432)          # query slice starts (block aligned)
Q_LEN = (832, 832, 768, 705)         # valid lengths (last includes cls @ 3136)


def _to_win(t):
    # [B,56,56,C] -> [B,3136,C] in (win_row, win_col, row_in_win, col_in_win) order
    t = t.reshape(B, NW, HWIN, NW, HWIN, C)
    return t.transpose(0, 1, 3, 2, 4, 5).reshape(B, S_SP, C)


def _from_win(t):
    # inverse of _to_win: [B,3136,C] -> [B,56,56,C]
    t = t.reshape(B, NW, NW, HWIN, HWIN, C)
    return t.transpose(0, 1, 3, 2, 4, 5).reshape(B, H, W, C)


def _lepe_host(x, Wp, bp, lepe_w, lepe_b):
    # depthwise 3x3 SAME conv on v (spatial layout), pure numpy
    Wv = Wp[2 * C:3 * C]
    bv = bp[2 * C:3 * C]
    v = x.reshape(B * S_SP, C) @ Wv.T + bv
    v = v.reshape(B, H, W, C)
    vp = np.zeros((B, H + 2, W + 2, C), dtype=v.dtype)
    vp[:, 1:-1, 1:-1] = v
    out = np.broadcast_to(lepe_b, (B, H, W, C)).copy()
    for dy in range(3):
        for dx in range(3):
            out += vp[:, dy:dy + H, dx:dx + W] * lepe_w[dy, dx, 0]
    return out  # [B,56,56,C]


def _attn_core(xw_all, mask_b, q_lo, b_idx, lepe_sl, Wp, bp, Wo, bo):
    # xw_all: [B,S,C] windowed tokens (+cls last); runs fully on one NeuronCore
    xw = xw_all[b_idx]
    qkv = xw @ Wp.T + bp                       # [S, 3C]
    q, k, v = jnp.split(qkv, 3, axis=-1)
    # pad so the largest q_lo stays in-bounds (dynamic_slice clamps OOB starts)
    qp = jnp.pad(q, ((0, Q_LO[-1] + QPAD - S), (0, 0)))
    q = jax.lax.dynamic_slice(qp, (q_lo, 0), (QPAD, C)) * SCALE
    qh = q.reshape(QPAD, NH, HD).transpose(1, 0, 2)      # [NH,QPAD,HD]
    kh = k.reshape(S, NH, HD).transpose(1, 0, 2)         # [NH,S,HD]
    vh = v.reshape(S, NH, HD).transpose(1, 0, 2)
    scores = jnp.einsum('nqd,nkd->nqk', qh, kh)          # [NH,QPAD,S]
    blk_q = jnp.clip((q_lo + jnp.arange(QPAD)) // BLK, 0, N_BLK - 1)
    blk_k = jnp.arange(S) // BLK
    allow = mask_b[blk_q][:, blk_k] == 1                 # [QPAD,S]
    scores = jnp.where(allow[None], scores, jnp.finfo(scores.dtype).min)
    attn = jax.nn.softmax(scores, axis=-1)
    out = jnp.einsum('nqk,nkd->nqd', attn, vh)           # [NH,QPAD,HD]
    out = out.transpose(1, 0, 2).reshape(QPAD, C)
    return (out + lepe_sl) @ Wo.T + bo                   # [QPAD,C]


_pmapped = jax.pmap(
    _attn_core,
    in_axes=(None, 0, 0, 0, 0, None, None, None, None),
    static_broadcasted_argnums=(),
)


def kernel(x, clstoken, mask, Wp, bp, lepe_w, lepe_b, Wo, bo):
    x = np.asarray(x, dtype=np.float32)
    clstoken = np.asarray(clstoken, dtype=np.float32)
    mask = np.asarray(mask, dtype=np.int32)
    Wp = np.asarray(Wp, np.float32); bp = np.asarray(bp, np.float32)
    lepe_w = np.asarray(lepe_w, np.float32); lepe_b = np.asarray(lepe_b, np.float32)
    Wo = np.asarray(Wo, np.float32); bo = np.asarray(bo, np.float32)

    # host prep: window permutation + cls concat + lepe depthwise conv
    xw = np.concatenate([_to_win(x), clstoken], axis=1)        # [B,S,C]
    lepe = _lepe_host(x, Wp, bp, lepe_w, lepe_b)               # [B,56,56,C]
    lepe_w_ord = np.concatenate(
        [_to_win(lepe), np.zeros((B, 1, C), np.float32)], axis=1)  # cls row: no lepe

    # per-core stacking: core i -> batch i//4, query quarter i%4
    mask_s = np.empty((8, N_BLK, N_BLK), np.int32)
    qlo_s = np.empty((8,), np.int32)
    bidx_s = np.empty((8,), np.int32)
    lepe_s = np.zeros((8, QPAD, C), np.float32)
    for i in range(8):
        b, qq = i // 4, i % 4
        mask_s[i] = mask[b]
        qlo_s[i] = Q_LO[qq]
        bidx_s[i] = b
        n = Q_LEN[qq]
        lepe_s[i, :n] = lepe_w_ord[b, Q_LO[qq]:Q_LO[qq] + n]

    out = _pmapped(xw, mask_s, qlo_s, bidx_s, lepe_s, Wp, bp, Wo, bo)
    out = np.asarray(out)                                      # [8,QPAD,C]

    # gather: concat valid query slices per batch
    full = np.empty((B, S, C), np.float32)
    for i in range(8):
        b, qq = i // 4, i % 4
        n = Q_LEN[qq]
        full[b, Q_LO[qq]:Q_LO[qq] + n] = out[i, :n]
    x_out = _from_win(full[:, :S_SP])
    cls_out = full[:, S_SP:S].copy()
    return x_out, cls_out


# revision 6
# speedup vs baseline: 1.5723x; 1.5723x over previous
import os
os.environ["NEURON_CC_FLAGS"] = os.environ.get("NEURON_CC_FLAGS", "") + " --auto-cast none"
import numpy as np
import jax
import jax.numpy as jnp

jax.config.update("jax_default_matmul_precision", "highest")

# Problem constants (nn_BiLevelRoutingAttention): B=2, H=W=56, C=256
B, H, W, C = 2, 56, 56, 256
NH, HD = 8, 32
NW, HWIN = 7, 8          # 7x7 windows of 8x8 tokens
S_SP = H * W             # 3136 spatial tokens
S = S_SP + 1             # + cls token
BLK = HWIN * HWIN        # 64 tokens per block
N_BLK = 50               # 49 spatial blocks + cls block
SCALE = C ** (-0.5)      # 1/16
QPAD = 832               # 13 blocks worth of queries per core (padded)
SPAD = 2432 + QPAD       # padded token length so all dynamic slices are in-bounds
Q_LO = (0, 832, 1664, 2432)          # query slice starts (block aligned)
Q_LEN = (832, 832, 768, 705)         # valid lengths (last includes cls @ 3136)


def _from_win(t):
    # inverse window permutation: [B,3136,C] -> [B,56,56,C]
    t = t.reshape(B, NW, NW, HWIN, HWIN, C)
    return t.transpose(0, 1, 3, 2, 4, 5).reshape(B, H, W, C)


def _attn_core(x_b, cls_b, mask_b, q_lo, Wp, bp, lepe_w, lepe_b, Wo, bo):
    # x_b: [56,56,C] one batch; computes its query-quarter of the full output
    xf = x_b.reshape(S_SP, C)
    qkv = xf @ Wp.T + bp                         # [3136, 3C] (spatial row-major)
    qkv_c = cls_b @ Wp.T + bp                    # [1, 3C]
    v_sp = qkv[:, 2 * C:].reshape(H, W, C)       # spatial v for LePE

    # LePE: depthwise 3x3 SAME conv via 9 shifted adds (on-device, cheap)
    vp = jnp.pad(v_sp, ((1, 1), (1, 1), (0, 0)))
    lepe = lepe_b + sum(
        vp[dy:dy + H, dx:dx + W] * lepe_w[dy, dx, 0]
        for dy in range(3) for dx in range(3))   # [56,56,C]

    def to_win(t):  # [56,56,M] -> [3136,M]
        m = t.shape[-1]
        return t.reshape(NW, HWIN, NW, HWIN, m).transpose(0, 2, 1, 3, 4).reshape(S_SP, m)

    qkv_w = jnp.concatenate([to_win(qkv.reshape(H, W, 3 * C)), qkv_c], axis=0)  # [S,3C]
    q, k, v = jnp.split(qkv_w, 3, axis=-1)
    lepe_w_ord = jnp.concatenate([to_win(lepe), jnp.zeros((1, C))], axis=0)     # cls: no lepe

    # slice this core's queries (pad first: dynamic_slice clamps OOB starts)
    qp = jnp.pad(q, ((0, SPAD - S), (0, 0)))
    lp = jnp.pad(lepe_w_ord, ((0, SPAD - S), (0, 0)))
    q = jax.lax.dynamic_slice(qp, (q_lo, 0), (QPAD, C)) * SCALE
    lepe_sl = jax.lax.dynamic_slice(lp, (q_lo, 0), (QPAD, C))

    qh = q.reshape(QPAD, NH, HD).transpose(1, 0, 2)      # [NH,QPAD,HD]
    kh = k.reshape(S, NH, HD).transpose(1, 0, 2)         # [NH,S,HD]
    vh = v.reshape(S, NH, HD).transpose(1, 0, 2)
    scores = jnp.einsum('nqd,nkd->nqk', qh, kh)          # [NH,QPAD,S]
    blk_q = jnp.clip((q_lo + jnp.arange(QPAD)) // BLK, 0, N_BLK - 1)
    blk_k = jnp.arange(S) // BLK
    allow = mask_b[blk_q][:, blk_k] == 1                 # [QPAD,S]
    scores = jnp.where(allow[None], scores, jnp.finfo(scores.dtype).min)
    attn = jax.nn.softmax(scores, axis=-1)
    out = jnp.einsum('nqk,nkd->nqd', attn, vh)           # [NH,QPAD,HD]
    out = out.transpose(1, 0, 2).reshape(QPAD, C)
    return (out + lepe_sl) @ Wo.T + bo                   # [QPAD,C]


_pmapped = jax.pmap(
    _attn_core,
    in_axes=(0, 0, 0, 0, None, None, None, None, None, None),
)


def kernel(x, clstoken, mask, Wp, bp, lepe_w, lepe_b, Wo, bo):
    x = np.asarray(x, dtype=np.float32)
    clstoken = np.asarray(clstoken, dtype=np.float32)
    mask = np.asarray(mask, dtype=np.int32)
    Wp = np.asarray(Wp, np.float32); bp = np.asarray(bp, np.float32)
    lepe_w = np.asarray(lepe_w, np.float32); lepe_b = np.asarray(lepe_b, np.float32)
    Wo = np.asarray(Wo, np.float32); bo = np.asarray(bo, np.float32)

    # per-core stacking: core i -> batch i//4, query quarter i%4
    idx = [i // 4 for i in range(8)]
    x_s = x[idx]                                  # [8,56,56,C]
    cls_s = clstoken[idx]                         # [8,1,C]
    mask_s = mask[idx]                            # [8,50,50]
    qlo_s = np.array([Q_LO[i % 4] for i in range(8)], np.int32)

    out = np.asarray(_pmapped(x_s, cls_s, mask_s, qlo_s,
                              Wp, bp, lepe_w, lepe_b, Wo, bo))  # [8,QPAD,C]

    # gather: concat valid query slices per batch
    full = np.empty((B, S, C), np.float32)
    for i in range(8):
        b, qq = i // 4, i % 4
        n = Q_LEN[qq]
        full[b, Q_LO[qq]:Q_LO[qq] + n] = out[i, :n]
    x_out = _from_win(full[:, :S_SP])
    cls_out = full[:, S_SP:S].copy()
    return x_out, cls_out
